# revision 1
# baseline (speedup 1.0000x reference)
"""DiT forward kernel for 8x Trainium2 NeuronCores (Bass/Tile).

Sharding: data-parallel over batch. Core b computes batch element b end to
end; weights are replicated (host-cast to bf16) across cores.

Layouts per core:
  - residual stream xc: tok-major fp32 SBUF [128, 2, 512]
      (partition = token % 128, chunk = token // 128, free = channel)
  - matmul activations: chan-major bf16 [128, S, 256]
      (partition = chan % 128, sub = chan // 128, free = token)
  - weights W^T staged [128, S, N] bf16 (partition = in-chan % 128,
      sub = in-chan // 128, free = out-chan), so matmul lhsT/rhs slices are
      direct partition tiles.

Attention is computed key-major: sT = k q^T, the pe-mask is built directly
transposed, softmax runs un-normalized (exp without max subtraction -- the
scores here are O(1)), and the normalizer Z comes from a ones-column
appended to v; the divide is fused into the per-head output copy.
"""

import math
import os
import sys

import numpy as np

try:
    import concourse.bass as bass
except Exception:
    sys.path.insert(0, "/opt/trn_rl_repo")
    import concourse.bass as bass

import ml_dtypes
from contextlib import ExitStack

import concourse.mybir as mybir
import concourse.tile as tile
from concourse import bacc
from concourse.bass_utils import run_bass_kernel_spmd

BF16 = mybir.dt.bfloat16
F32 = mybir.dt.float32
AF = mybir.ActivationFunctionType
ALU = mybir.AluOpType
AX = mybir.AxisListType
ts = bass.ts

P = 128
B, TOK, HID, LAT, OUT_C, HEADS = 8, 256, 128, 512, 256, 8
DEPTH = int(os.environ.get("DIT_DEPTH", "36"))
MLP_H = 2048
HD = LAT // HEADS  # 64
FREQ = 256
TC = TOK // P      # 2 token chunks
LS = LAT // P      # 4 channel subtiles
MS = MLP_H // P    # 16
PI = math.pi
TWO_PI = 2.0 * math.pi


# ---------------------------------------------------------------------------
# device program helpers
# ---------------------------------------------------------------------------


def _range_reduce(nc, pool, x_ap, shape, tag):
    """x >= 0 -> x mod 2pi folded into [-pi, pi), via int cast round/trunc."""
    t = pool.tile(shape, F32, tag=f"rr_t_{tag}")
    nc.vector.tensor_scalar(t[:], x_ap, 1.0 / TWO_PI, None, ALU.mult)
    ti = pool.tile(shape, mybir.dt.int32, tag=f"rr_i_{tag}")
    nc.vector.tensor_copy(ti[:], t[:])
    nc.vector.tensor_copy(t[:], ti[:])
    red = pool.tile(shape, F32, tag=f"rr_r_{tag}")
    nc.vector.scalar_tensor_tensor(red[:], t[:], -TWO_PI, x_ap, ALU.mult,
                                   ALU.add)
    nc.vector.tensor_scalar(t[:], red[:], PI, None, ALU.is_ge)
    nc.vector.scalar_tensor_tensor(red[:], t[:], -TWO_PI, red[:], ALU.mult,
                                   ALU.add)
    return red


def _ln_stats(nc, small, src_ap, eps_ap):
    """Free-dim LayerNorm stats of [128, N] fp32 -> (rstd, -mean*rstd)."""
    bnst = small.tile([P, 6], F32, tag="bnst")
    nc.vector.bn_stats(bnst[:], src_ap)
    mv = small.tile([P, 2], F32, tag="mv")
    nc.vector.bn_aggr(mv[:], bnst[:])
    sd = small.tile([P, 1], F32, tag="sd")
    nc.scalar.activation(sd[:], mv[:, 1:2], AF.Ln, bias=eps_ap)
    r = small.tile([P, 1], F32, tag="rstd")
    nc.scalar.activation(r[:], sd[:], AF.Exp, scale=-0.5)
    mb = small.tile([P, 1], F32, tag="mb")
    nc.vector.tensor_scalar(mb[:], mv[:, 0:1], r[:], -1.0, ALU.mult, ALU.mult)
    return r, mb


def _ln_modulate_transpose(nc, small, xn_pool, psum, ident, xc, sc_ap, sh_ap,
                           eps_ap, hhT, tag):
    """LayerNorm(tok-major fp32 xc) -> chan-major, *(1+sc)+sh -> bf16 hhT."""
    xn = xn_pool.tile([P, TC, LAT], BF16, tag=f"xn_{tag}")
    for c in range(TC):
        r, mb = _ln_stats(nc, small, xc[:, c, :], eps_ap)
        nc.scalar.activation(xn[:, c, :], xc[:, c, :], AF.Identity,
                             bias=mb[:], scale=r[:])
    for c in range(TC):
        for s in range(LS):
            pst = psum.tile([P, P], BF16, tag="ps")
            nc.tensor.transpose(pst[:], xn[:, c, ts(s, P)], ident[:])
            nc.vector.tensor_scalar(
                hhT[:, s, ts(c, P)], pst[:],
                sc_ap[:, s:s + 1], sh_ap[:, s:s + 1], ALU.mult, ALU.add)


def build_program(depth=DEPTH):
    nc = bacc.Bacc("TRN2", target_bir_lowering=False, debug=False,
                   num_devices=8)

    def din(name, shape, dt):
        return nc.dram_tensor(name, list(shape), dt,
                              kind="ExternalInput").ap()

    x_t = din("x_t", [P, TC, HID], F32)
    coords_t = din("coords_t", [P, TC, 3], F32)
    ident_d = din("ident", [P, P], BF16)
    div_bc_d = din("div_bc", [P, TC, 256], F32)
    cvec_d = din("cvec", [P, 8], F32)
    projw_d = din("projw", [P, 1, LAT], BF16)
    te1_d = din("te1", [P, 2, LAT], BF16)
    te2_d = din("te2", [P, LS, LAT], BF16)
    # early group: [ada | p1 | p2 | qk | v | pj] along free dim
    WE = 6 * LAT + LAT + LAT + 2 * LAT + LAT + LAT  # per-sub free elems
    we_d = din("we_w", [depth, P, LS, WE], BF16)
    # late group: [f1 (LS subs x MLP_H) | f2 (MS subs x LAT)]
    WL = LS * MLP_H + MS * LAT
    wl_d = din("wl_w", [depth, P, WL], BF16)
    finada_d = din("finada", [P, LS, 2 * LAT], BF16)
    finw_d = din("finw", [P, LS, OUT_C], BF16)

    out_t = nc.dram_tensor("out_t", [P, TC, OUT_C], F32,
                           kind="ExternalOutput").ap()

    with tile.TileContext(nc) as tc:
        with ExitStack() as ctx:
            _emit(ctx, tc, nc, depth, x_t, coords_t, ident_d,
                  div_bc_d, cvec_d, projw_d, te1_d, te2_d,
                  we_d, wl_d, finada_d, finw_d, out_t)
    nc.compile()
    return nc


def _emit(ctx, tc, nc, depth, x_t, coords_t, ident_d, div_bc_d,
          cvec_d, projw_d, te1_d, te2_d, we_d, wl_d, finada_d, finw_d,
          out_t):
    def pool(name, bufs, space="SBUF"):
        return ctx.enter_context(
            tc.tile_pool(name=name, bufs=bufs, space=space))

    pers = pool("pers", 1)    # persistent state + consts
    once = pool("once", 1)    # prelude / final-layer temporaries
    wq = pool("wq", 1)        # per-block weights (single-buffered, see note)
    blk1 = pool("blk1", 1)    # within-block activations
    blk2 = pool("blk2", 2)    # cross-block pipelined activations
    head3 = pool("head3", 2)  # per-head-pair attention temporaries
    small = pool("small", 3)  # tiny stat tiles
    psum = pool("psum", 5, space="PSUM")
    psrow = pool("psrow", 1, space="PSUM")
    dram = pool("dram", 2, space="DRAM")

    # ---------------- persistent/consts ----------------
    ident = pers.tile([P, P], BF16, tag="ident")
    nc.sync.dma_start(ident[:], ident_d)
    cvec = pers.tile([P, 8], F32, tag="cvec")
    nc.sync.dma_start(cvec[:], cvec_d)
    negpi = cvec[:, 0:1]
    eps5 = cvec[:, 1:2]
    eps6 = cvec[:, 2:3]
    sel2 = cvec[:, 5:7]

    xc = pers.tile([P, TC, LAT], F32, tag="xc")
    v_aug = pers.tile([P, TC, HEADS, HD + 1], BF16, tag="v_aug")
    nc.vector.memset(v_aug[:, :, :, HD:HD + 1], 1.0)

    # ---------------- t embedding -> sT = silu(c) chan-major ----------------
    tf = small.tile([P, 1], F32, tag="tf")
    nc.vector.tensor_scalar(tf[:], cvec[:, 3:4], cvec[:, 4:5], None,
                            ALU.mult)
    embT = once.tile([P, 2, 1], BF16, tag="embT")
    for idx, off in ((0, PI / 2.0), (1, 0.0)):  # sub0=cos, sub1=sin
        xsh = small.tile([P, 1], F32, tag="tf_sh")
        nc.vector.tensor_scalar(xsh[:], tf[:], off, None, ALU.add)
        red = _range_reduce(nc, small, xsh[:], [P, 1], "emb")
        nc.scalar.activation(embT[:, idx, :], red[:], AF.Sin)
    te1 = once.tile([P, 2, LAT], BF16, tag="te1")
    nc.sync.dma_start(te1[:], te1_d)
    te2 = once.tile([P, LS, LAT], BF16, tag="te2")
    nc.sync.dma_start(te2[:], te2_d)
    ps_h1 = psrow.tile([1, LAT], F32, tag="ps_row")
    for s in range(2):
        nc.tensor.matmul(ps_h1[:], embT[:, s, :], te1[:, s, :],
                         start=(s == 0), stop=(s == 1))
    h1row = once.tile([1, LAT], BF16, tag="h1row")
    nc.scalar.activation(h1row[:], ps_h1[:], AF.Silu)
    h1_dr = dram.tile([1, LAT], BF16, tag="h1_dr")
    nc.sync.dma_start(h1_dr[:], h1row[:])
    h1T = once.tile([P, LS, 1], BF16, tag="h1T")
    nc.sync.dma_start(h1T[:, :, 0],
                      h1_dr[:].rearrange("o (s p) -> (o p) s", s=LS, p=P))
    ps_c = psrow.tile([1, LAT], F32, tag="ps_row")
    for s in range(LS):
        nc.tensor.matmul(ps_c[:], h1T[:, s, :], te2[:, s, :],
                         start=(s == 0), stop=(s == LS - 1))
    s_row = once.tile([1, LAT], BF16, tag="s_row")
    nc.scalar.activation(s_row[:], ps_c[:], AF.Silu)
    s_dr = dram.tile([1, LAT], BF16, tag="s_dr")
    nc.sync.dma_start(s_dr[:], s_row[:])
    sT = pers.tile([P, LS, 1], BF16, tag="sT")
    nc.sync.dma_start(sT[:, :, 0],
                      s_dr[:].rearrange("o (s p) -> (o p) s", s=LS, p=P))

    # ---------------- positional encoding ----------------
    cds = once.tile([P, TC, 3], F32, tag="cds")
    nc.sync.dma_start(cds[:], coords_t)
    div_bc = once.tile([P, TC, 256], F32, tag="div_bc")
    nc.sync.dma_start(div_bc[:], div_bc_d)
    enc = small.tile([P, TC, 1], F32, tag="enc")
    nc.vector.scalar_tensor_tensor(enc[:, :, 0], cds[:, :, 1], 100.0,
                                   cds[:, :, 2], ALU.mult, ALU.add)
    nc.vector.scalar_tensor_tensor(enc[:, :, 0], cds[:, :, 0], 10000.0,
                                   enc[:, :, 0], ALU.mult, ALU.add)
    ang = once.tile([P, TC, 256], F32, tag="ang")
    nc.vector.tensor_tensor(ang[:], div_bc[:],
                            enc[:].to_broadcast((P, TC, 256)), ALU.mult)
    pe = pers.tile([P, TC, LAT], F32, tag="pe")
    pe4 = pe[:].rearrange("p c (j k) -> p c j k", j=256, k=2)
    for k, off in ((0, 0.0), (1, PI / 2.0)):  # even=sin, odd=cos
        xsh = once.tile([P, TC, 256], F32, tag="ang_sh")
        nc.vector.tensor_scalar(xsh[:], ang[:], off, None, ALU.add)
        red = _range_reduce(nc, once, xsh[:], [P, TC, 256], "pe")
        nc.scalar.activation(pe4[:, :, :, k], red[:], AF.Sin)
    peb = once.tile([P, TC, LAT], BF16, tag="peb")
    nc.vector.tensor_copy(peb[:], pe[:])
    peT = pers.tile([P, LS, TOK], BF16, tag="peT")
    for c in range(TC):
        for s in range(LS):
            pst = psum.tile([P, P], BF16, tag="ps")
            nc.tensor.transpose(pst[:], peb[:, c, ts(s, P)], ident[:])
            nc.vector.tensor_copy(peT[:, s, ts(c, P)], pst[:])

    # ---------------- input projection ----------------
    xin = once.tile([P, TC, HID], F32, tag="xin")
    nc.sync.dma_start(xin[:], x_t)
    xn0 = once.tile([P, TC, HID], BF16, tag="xn0")
    for c in range(TC):
        r, mb = _ln_stats(nc, small, xin[:, c, :], eps5)
        nc.scalar.activation(xn0[:, c, :], xin[:, c, :], AF.Identity,
                             bias=mb[:], scale=r[:])
    xn0T = once.tile([P, TOK], BF16, tag="xn0T")
    for c in range(TC):
        pst = psum.tile([P, P], BF16, tag="ps")
        nc.tensor.transpose(pst[:], xn0[:, c, :], ident[:])
        nc.vector.tensor_copy(xn0T[:, ts(c, P)], pst[:])
    projw = once.tile([P, 1, LAT], BF16, tag="projw")
    nc.sync.dma_start(projw[:], projw_d)
    for c in range(TC):
        ph = psum.tile([P, LAT], F32, tag="ps")
        nc.tensor.matmul(ph[:], xn0T[:, ts(c, P)], projw[:, 0, :],
                         start=True, stop=True)
        hsb = once.tile([P, LAT], F32, tag="h_sb")
        nc.scalar.activation(hsb[:], ph[:], AF.Copy)
        r, mb = _ln_stats(nc, small, hsb[:], eps5)
        hn = once.tile([P, LAT], F32, tag="hn")
        nc.scalar.activation(hn[:], hsb[:], AF.Identity, bias=mb[:],
                             scale=r[:])
        nc.vector.tensor_tensor(xc[:, c, :], hn[:], pe[:, c, :], ALU.add)

    # ---------------- transformer blocks ----------------
    inv_gn = 1.0 / (64.0 * 256.0)

    for d in range(depth):
        # --- block weights: 2 grouped DMAs (early / late by use time) ---
        WE = 12 * LAT
        wet = wq.tile([P, LS, WE], BF16, tag="wet")
        nc.sync.dma_start(wet[:], we_d[d])
        adaw = wet[:, :, 0:6 * LAT]
        p1w = wet[:, :, 6 * LAT:7 * LAT]
        p2w = wet[:, :, 7 * LAT:8 * LAT]
        qkw = wet[:, :, 8 * LAT:10 * LAT]
        vw = wet[:, :, 10 * LAT:11 * LAT]
        pjw = wet[:, :, 11 * LAT:12 * LAT]
        wlt = wq.tile([P, LS * MLP_H + MS * LAT], BF16, tag="wlt")
        nc.sync.dma_start(wlt[:], wl_d[d])
        f1w = wlt[:, 0:LS * MLP_H].rearrange("p (s n) -> p s n", s=LS)
        f2w = wlt[:, LS * MLP_H:].rearrange("p (s n) -> p s n", s=MS)

        # --- ada = silu(c) @ aw^T : [1, 3072] row, then scatter via DRAM ---
        ada_dr = dram.tile([1, 6 * LAT], BF16, tag="ada_dr")
        for n2 in range(2):
            psa = psrow.tile([1, 3 * LAT], F32, tag="ps_row")
            for n in range(3):
                for s in range(LS):
                    nc.tensor.matmul(psa[:, ts(n, LAT)], sT[:, s, :],
                                     adaw[:, s, ts(3 * n2 + n, LAT)],
                                     start=(s == 0), stop=(s == LS - 1))
            arow = blk1.tile([1, 3 * LAT], BF16, tag="ada_row")
            nc.scalar.activation(arow[:], psa[:], AF.Copy)
            nc.sync.dma_start(ada_dr[:, ts(n2, 3 * LAT)], arow[:])
        adaTb = blk2.tile([P, 6, LS], BF16, tag="adaTb")
        nc.sync.dma_start(
            adaTb[:],
            ada_dr[:].rearrange("o (v s p) -> (o p) v s", v=6, s=LS, p=P))
        adaT = blk2.tile([P, 6, LS], F32, tag="adaT")
        nc.vector.tensor_copy(adaT[:], adaTb[:])
        nc.vector.tensor_scalar(adaT[:, 1, :], adaT[:, 1, :], 1.0, None,
                                ALU.add)
        nc.vector.tensor_scalar(adaT[:, 4, :], adaT[:, 4, :], 1.0, None,
                                ALU.add)
        ga_bc = blk2.tile([P, LAT], BF16, tag="ga_bc")
        nc.sync.dma_start(ga_bc[:],
                          ada_dr[:, ts(2, LAT)].to_broadcast((P, LAT)))
        gm_bc = blk2.tile([P, LAT], BF16, tag="gm_bc")
        nc.sync.dma_start(gm_bc[:],
                          ada_dr[:, ts(5, LAT)].to_broadcast((P, LAT)))

        # --- mask chain: pe1/pe2 matmuls + groupnorm + maskT + sigmoid ---
        def pe_branch(w_sb, dst_bf, tagp):
            raw = blk2.tile([P, LS, TOK], BF16, tag=f"pe_raw_{tagp}")
            st1 = small.tile([P, 8], F32, tag="gn_st1")
            for m in range(LS):
                psp = psum.tile([P, TOK], F32, tag="ps")
                for s in range(LS):
                    nc.tensor.matmul(psp[:], w_sb[:, s, ts(m, P)],
                                     peT[:, s, :], start=(s == 0),
                                     stop=(s == LS - 1))
                nc.vector.tensor_reduce(st1[:, m:m + 1], psp[:], axis=AX.X,
                                        op=ALU.add)
                sq = blk1.tile([P, TOK], BF16, tag="gn_sq")
                nc.scalar.activation(sq[:], psp[:], AF.Square,
                                     accum_out=st1[:, 4 + m:5 + m])
                nc.vector.tensor_copy(raw[:, m, :], psp[:])
            gs = psum.tile([4, 4], F32, tag="ps")
            nc.tensor.matmul(gs[:, 0:2], st1[:, 0:4], sel2, start=True,
                             stop=True)
            nc.tensor.matmul(gs[:, 2:4], st1[:, 4:8], sel2, start=True,
                             stop=True)
            mu = small.tile([4, 2], F32, tag="gn_mu")
            nc.vector.tensor_scalar(mu[:], gs[:, 0:2], inv_gn, None,
                                    ALU.mult)
            m2 = small.tile([4, 2], F32, tag="gn_m2")
            nc.vector.tensor_scalar(m2[:], gs[:, 2:4], inv_gn, None,
                                    ALU.mult)
            msq = small.tile([4, 2], F32, tag="gn_msq")
            nc.scalar.activation(msq[:], mu[:], AF.Square)
            var = small.tile([4, 2], F32, tag="gn_var")
            nc.vector.tensor_tensor(var[:], m2[:], msq[:], ALU.subtract)
            sd = small.tile([4, 2], F32, tag="gn_sd")
            nc.scalar.activation(sd[:], var[:], AF.Ln, bias=eps5[0:4, :])
            ab = small.tile([4, 2, 2], F32, tag="gn_ab")
            nc.scalar.activation(ab[:, :, 0], sd[:], AF.Exp, scale=-0.5)
            nc.vector.scalar_tensor_tensor(ab[:, :, 1], mu[:], -1.0,
                                           ab[:, :, 0], ALU.mult, ALU.mult)
            gn_d = dram.tile([1, 16], F32, tag="gn_d")
            nc.sync.dma_start(
                gn_d[:].rearrange("o (s j k) -> (o s) j k", s=4, j=2, k=2),
                ab[:])
            abbc = small.tile([P, LS, 2], F32, tag="gn_abbc")
            g3 = gn_d[:].rearrange("o (s j k) -> (o j) s k", s=4, j=2, k=2)
            for j in range(2):
                nc.sync.dma_start(
                    abbc[j * 64:(j + 1) * 64, :, :],
                    g3[j:j + 1, :, :].to_broadcast((64, 4, 2)))
            for m in range(LS):
                nc.vector.tensor_scalar(
                    dst_bf[:, m, :], raw[:, m, :],
                    abbc[:, m, 0:1], abbc[:, m, 1:2], ALU.mult, ALU.add)

        pe1n = blk2.tile([P, LS, TOK], BF16, tag="pe1n")
        pe_branch(p1w, pe1n[:], "p1")
        pe2n = blk2.tile([P, LS, TOK], BF16, tag="pe2n")
        pe_branch(p2w, pe2n[:], "p2")
        maskT = blk2.tile([P, TC, TOK], F32, tag="maskT")
        for mc in range(TC):
            psm = psum.tile([P, TOK], F32, tag="ps")
            for s in range(LS):
                nc.tensor.matmul(psm[:], pe2n[:, s, ts(mc, P)],
                                 pe1n[:, s, :], start=(s == 0),
                                 stop=(s == LS - 1))
            # sigmoid(x) = 1 / (1 + exp(-x)) -- stays in the exp table set
            en = blk1.tile([P, TOK], F32, tag="mask_en")
            nc.scalar.activation(en[:], psm[:], AF.Exp, scale=-1.0)
            nc.vector.tensor_scalar(en[:], en[:], 1.0, None, ALU.add)
            nc.vector.reciprocal(maskT[:, mc, :], en[:])

        # --- LN1 + modulate + transpose ---
        hh1T = blk1.tile([P, LS, TOK], BF16, tag="hh1T")
        _ln_modulate_transpose(nc, small, blk1, psum, ident, xc[:],
                               adaT[:, 1, :], adaT[:, 0, :], eps6, hh1T[:],
                               "l1")

        # --- qk^T (chan-major) and v (tok-major, ones col appended) ---
        qkT = blk1.tile([P, 2 * LS, TOK], BF16, tag="qkT")
        for m in range(2 * LS):
            psq = psum.tile([P, TOK], F32, tag="ps")
            for s in range(LS):
                nc.tensor.matmul(psq[:], qkw[:, s, ts(m, P)], hh1T[:, s, :],
                                 start=(s == 0), stop=(s == LS - 1))
            nc.scalar.activation(qkT[:, m, :], psq[:], AF.Copy)
        for c in range(TC):
            psv = psum.tile([P, LAT], F32, tag="ps")
            for s in range(LS):
                nc.tensor.matmul(psv[:], hh1T[:, s, ts(c, P)], vw[:, s, :],
                                 start=(s == 0), stop=(s == LS - 1))
            nc.scalar.activation(
                v_aug[:, c, :, 0:HD],
                psv[:].rearrange("p (h d) -> p h d", h=HEADS), AF.Copy)

        # --- attention, key-major scores ---
        attn = blk1.tile([P, TC, LAT], BF16, tag="attn")
        for hp in range(HEADS // 2):
            stf = head3.tile([P, 2, TC, TOK], BF16, tag="stf")
            for i in range(2):
                h = 2 * hp + i
                pbase = (h % 2) * HD
                qs = qkT[pbase:pbase + HD, h // 2, :]
                ks = qkT[pbase:pbase + HD, 4 + h // 2, :]
                pss = psum.tile([P, TC, TOK], F32, tag="ps")
                for kc in range(TC):
                    nc.tensor.matmul(pss[:, kc, :], ks[:, ts(kc, P)], qs,
                                     start=True, stop=True)
                nc.vector.tensor_tensor(stf[:, i], pss[:], maskT[:],
                                        ALU.mult)
            ptil = head3.tile([P, 2, TC, TOK], BF16, tag="ptil")
            nc.scalar.activation(ptil[:], stf[:], AF.Exp)
            for i in range(2):
                h = 2 * hp + i
                for qc in range(TC):
                    pso = psum.tile([P, HD + 1], F32, tag="ps")
                    for kc in range(TC):
                        nc.tensor.matmul(pso[:], ptil[:, i, kc, ts(qc, P)],
                                         v_aug[:, kc, h, :],
                                         start=(kc == 0),
                                         stop=(kc == TC - 1))
                    rz = small.tile([P, 1], F32, tag="rz")
                    nc.vector.reciprocal(rz[:], pso[:, HD:HD + 1])
                    nc.vector.tensor_scalar(attn[:, qc, ts(h, HD)],
                                            pso[:, 0:HD], rz[:], None,
                                            ALU.mult)
        attnT = blk1.tile([P, LS, TOK], BF16, tag="attnT")
        for c in range(TC):
            for s in range(LS):
                pst = psum.tile([P, P], BF16, tag="ps")
                nc.tensor.transpose(pst[:], attn[:, c, ts(s, P)], ident[:])
                nc.scalar.activation(attnT[:, s, ts(c, P)], pst[:], AF.Copy)

        # --- attn proj + gated residual ---
        for c in range(TC):
            psp = psum.tile([P, LAT], F32, tag="ps")
            for s in range(LS):
                nc.tensor.matmul(psp[:], attnT[:, s, ts(c, P)], pjw[:, s, :],
                                 start=(s == 0), stop=(s == LS - 1))
            gated = blk2.tile([P, LAT], F32, tag="gated")
            nc.vector.tensor_tensor(gated[:], psp[:], ga_bc[:], ALU.mult)
            nc.vector.tensor_tensor(xc[:, c, :], xc[:, c, :], gated[:],
                                    ALU.add)

        # --- LN2 + modulate + transpose ---
        hh2T = blk1.tile([P, LS, TOK], BF16, tag="hh2T")
        _ln_modulate_transpose(nc, small, blk1, psum, ident, xc[:],
                               adaT[:, 4, :], adaT[:, 3, :], eps6, hh2T[:],
                               "l2")

        # --- MLP ---
        mlpT = blk1.tile([P, MS, TOK], BF16, tag="mlpT")
        for mp in range(MS // 2):
            psf = psum.tile([P, 2, TOK], F32, tag="ps")
            for half in range(2):
                m = 2 * mp + half
                for s in range(LS):
                    nc.tensor.matmul(psf[:, half, :], f1w[:, s, ts(m, P)],
                                     hh2T[:, s, :], start=(s == 0),
                                     stop=(s == LS - 1))
            nc.scalar.activation(mlpT[:, 2 * mp:2 * mp + 2, :], psf[:],
                                 AF.Gelu)
        for c in range(TC):
            psm = psum.tile([P, LAT], F32, tag="ps")
            for s in range(MS):
                nc.tensor.matmul(psm[:], mlpT[:, s, ts(c, P)], f2w[:, s, :],
                                 start=(s == 0), stop=(s == MS - 1))
            gated = blk2.tile([P, LAT], F32, tag="gated")
            nc.vector.tensor_tensor(gated[:], psm[:], gm_bc[:], ALU.mult)
            nc.vector.tensor_tensor(xc[:, c, :], xc[:, c, :], gated[:],
                                    ALU.add)

    # ---------------- final layer ----------------
    finada = once.tile([P, LS, 2 * LAT], BF16, tag="finada")
    nc.sync.dma_start(finada[:], finada_d)
    finw = once.tile([P, LS, OUT_C], BF16, tag="finw")
    nc.sync.dma_start(finw[:], finw_d)
    adaf_dr = dram.tile([1, 2 * LAT], F32, tag="adaf_dr")
    for n in range(2):
        psa = psrow.tile([1, LAT], F32, tag="ps_row")
        for s in range(LS):
            nc.tensor.matmul(psa[:], sT[:, s, :], finada[:, s, ts(n, LAT)],
                             start=(s == 0), stop=(s == LS - 1))
        adaf_row = once.tile([1, LAT], F32, tag="adaf_row")
        nc.scalar.activation(adaf_row[:], psa[:], AF.Copy)
        nc.sync.dma_start(adaf_dr[:, ts(n, LAT)], adaf_row[:])
    adaTf = once.tile([P, 2, LS], F32, tag="adaTf")
    nc.sync.dma_start(
        adaTf[:],
        adaf_dr[:].rearrange("o (v s p) -> (o p) v s", v=2, s=LS, p=P))
    nc.vector.tensor_scalar(adaTf[:, 1, :], adaTf[:, 1, :], 1.0, None,
                            ALU.add)
    hhfT = once.tile([P, LS, TOK], BF16, tag="hhfT")
    _ln_modulate_transpose(nc, small, once, psum, ident, xc[:],
                           adaTf[:, 1, :], adaTf[:, 0, :], eps6, hhfT[:],
                           "lf")
    outsb = once.tile([P, TC, OUT_C], F32, tag="outsb")
    for mc in range(TC):
        pso = psum.tile([P, OUT_C], F32, tag="ps")
        for s in range(LS):
            nc.tensor.matmul(pso[:], finw[:, s, ts(mc, P)], hhfT[:, s, :],
                             start=(s == 0), stop=(s == LS - 1))
        nc.scalar.activation(outsb[:, mc, :], pso[:], AF.Copy)
    nc.sync.dma_start(out_t, outsb[:])


# ---------------------------------------------------------------------------
# host side
# ---------------------------------------------------------------------------

def _to_bf16(a):
    return np.asarray(a, dtype=np.float32).astype(ml_dtypes.bfloat16)


def _stage_wT(w, S, N):
    """w: [..., N_out, K] -> W^T staged [..., 128, S, N_out] bf16."""
    wt = np.ascontiguousarray(np.swapaxes(np.asarray(w, np.float32), -1, -2))
    shp = wt.shape
    K, M = shp[-2], shp[-1]
    assert K == S * P and M == N, (shp, S, N)
    wt = wt.reshape(shp[:-2] + (S, P, M))
    wt = np.swapaxes(wt, -3, -2)  # [..., P, S, M]
    return _to_bf16(np.ascontiguousarray(wt))


_CACHE = {}


def _get_program():
    if DEPTH not in _CACHE:
        _CACHE[DEPTH] = build_program(DEPTH)
    return _CACHE[DEPTH]


def prepare_in_maps(inputs, depth=DEPTH):
    f32 = np.float32
    x = np.asarray(inputs["x"], f32)
    t = np.asarray(inputs["t"], f32)
    coords = np.asarray(inputs["coords"], f32)

    qkv = np.asarray(inputs["blk_qkv_w"], f32)[:depth]
    qk = qkv[:, :2 * LAT, :].copy()
    qk[:, :LAT, :] *= HD ** -0.5
    vw = qkv[:, 2 * LAT:, :]

    common = {
        "ident": _to_bf16(np.eye(P)),
        "div_bc": np.ascontiguousarray(np.broadcast_to(
            np.exp(np.arange(0, LAT, 2, dtype=f32)
                   * (-math.log(10000.0) / LAT)),
            (P, TC, 256)).astype(f32)),
        "projw": _stage_wT(inputs["proj_w"], 1, LAT),
        "te1": _stage_wT(inputs["te_w1"], 2, LAT),
        "te2": _stage_wT(inputs["te_w2"], LS, LAT),
        "we_w": np.concatenate([
            _stage_wT(np.asarray(inputs["blk_ada_w"], f32)[:depth],
                      LS, 6 * LAT),
            _stage_wT(np.asarray(inputs["blk_pe1_w"], f32)[:depth], LS, LAT),
            _stage_wT(np.asarray(inputs["blk_pe2_w"], f32)[:depth], LS, LAT),
            _stage_wT(qk, LS, 2 * LAT),
            _stage_wT(vw, LS, LAT),
            _stage_wT(np.asarray(inputs["blk_proj_w"], f32)[:depth],
                      LS, LAT),
        ], axis=3),
        "wl_w": np.concatenate([
            _stage_wT(np.asarray(inputs["blk_fc1_w"], f32)[:depth],
                      LS, MLP_H).reshape(depth, P, LS * MLP_H),
            _stage_wT(np.asarray(inputs["blk_fc2_w"], f32)[:depth],
                      MS, LAT).reshape(depth, P, MS * LAT),
        ], axis=2),
        "finada": _stage_wT(inputs["fin_ada_w"], LS, 2 * LAT),
        "finw": _stage_wT(inputs["fin_w"], LS, OUT_C),
    }
    in_maps = []
    for b in range(B):
        m = dict(common)
        m["x_t"] = np.ascontiguousarray(
            x[b].T.reshape(TC, P, HID).swapaxes(0, 1))
        m["coords_t"] = np.ascontiguousarray(
            coords[b].T.reshape(TC, P, 3).swapaxes(0, 1))
        cv = np.zeros((P, 8), f32)
        cv[:, 0] = -math.pi
        cv[:, 1] = 1e-5
        cv[:, 2] = 1e-6
        cv[:, 3] = np.exp(-math.log(10000.0)
                          * np.arange(FREQ // 2, dtype=f32) / (FREQ // 2))
        cv[:, 4] = t[b]
        cv[:, 5] = (np.arange(P) // 64 == 0)
        cv[:, 6] = (np.arange(P) // 64 == 1)
        m["cvec"] = cv
        in_maps.append(m)
    return in_maps


def run_spmd(inputs, **kw):
    nc = _get_program()
    in_maps = prepare_in_maps(inputs, DEPTH)
    res = run_bass_kernel_spmd(nc, in_maps, core_ids=list(range(B)), **kw)
    outs = []
    for b in range(B):
        o = np.asarray(res.results[b]["out_t"], np.float32)  # [P, TC, OUT]
        outs.append(o.transpose(1, 0, 2).reshape(OUT_C, TOK))
    return np.stack(outs, axis=0), res


def kernel(**inputs):
    out, _ = run_spmd(inputs)
    return out



# revision 2
# speedup vs baseline: 1.5816x; 1.5816x over previous
"""DiT forward kernel for 8x Trainium2 NeuronCores (Bass/Tile), v2.

Sharding: data-parallel over batch. Core b computes batch element b end to
end; weights are replicated (host-cast to bf16) across cores.

v2 structural changes vs the staged baseline:
  - per-block weights split into 6 use-ordered groups (ada / pe-mask /
    qkv / proj / fc1 / fc2), each in its own single-buffered pool so the
    next block's group DMA starts right after this block's last use --
    weight streaming overlaps compute instead of serializing at block
    boundaries.
  - LayerNorm rstd computed without AF.Ln: bitcast-log2 seed + AF.Exp +
    one Newton step. With the mask sigmoid rewritten as tanh (in the exp
    table set), every per-block ACT op except Gelu stays in the
    exp_and_others set: 2 table loads per block instead of ~15.
  - GroupNorm partition-reduction via a transposed stat matmul
    (sel2^T @ st1 -> [half, stats]) plus a PE broadcast matmul -- no
    DRAM round trip in the mask chain.
  - attention: scores stay key-major; softmax normalizer comes from the
    ones-column of v_aug, inverted with reciprocal_approx_fast and
    applied as one broadcast multiply per (q-chunk, head-group).
  - ada / mask chains only depend on per-block weights + static pe/sT,
    so the Tile scheduler runs them a block ahead of the serial xc
    chain.

Layouts per core (unchanged from baseline):
  - residual xc: tok-major fp32 [128, 2, 512] (partition = token % 128)
  - matmul activations: chan-major bf16 [128, S, 256]
  - weights W^T staged [128, S, N] bf16
"""

import math
import os
import sys

import numpy as np

try:
    import concourse.bass as bass
except Exception:
    sys.path.insert(0, "/opt/trn_rl_repo")
    import concourse.bass as bass

import ml_dtypes
from contextlib import ExitStack

import concourse.mybir as mybir
import concourse.tile as tile
from concourse import bacc
from concourse.bass_utils import run_bass_kernel_spmd

BF16 = mybir.dt.bfloat16
F32 = mybir.dt.float32
I32 = mybir.dt.int32
AF = mybir.ActivationFunctionType
ALU = mybir.AluOpType
AX = mybir.AxisListType
ts = bass.ts

P = 128
B, TOK, HID, LAT, OUT_C, HEADS = 8, 256, 128, 512, 256, 8
DEPTH = int(os.environ.get("DIT_DEPTH", "36"))
MLP_H = 2048
HD = LAT // HEADS  # 64
FREQ = 256
TC = TOK // P      # 2 token chunks
LS = LAT // P      # 4 channel subtiles
MS = MLP_H // P    # 16
PI = math.pi
TWO_PI = 2.0 * math.pi
LN2 = math.log(2.0)
# bitcast-log2 rsqrt seed: for v>0, float(bitcast_i32(v)) = 2^23*(e+127+m),
# log2(v) ~ float(i)/2^23 - 127 + SIG0.  l = -0.5*log2(v):
SIG0 = 0.0430357
RS_SCALE = -0.5 / (1 << 23)
RS_BIAS = 0.5 * (127.0 - SIG0)


# ---------------------------------------------------------------------------
# device program helpers
# ---------------------------------------------------------------------------


def _range_reduce(nc, pool, x_ap, shape, tag):
    """x >= 0 -> x mod 2pi folded into [-pi, pi), via int cast round/trunc."""
    t = pool.tile(shape, F32, tag=f"rr_t_{tag}")
    nc.vector.tensor_scalar(t[:], x_ap, 1.0 / TWO_PI, None, ALU.mult)
    ti = pool.tile(shape, mybir.dt.int32, tag=f"rr_i_{tag}")
    nc.vector.tensor_copy(ti[:], t[:])
    nc.vector.tensor_copy(t[:], ti[:])
    red = pool.tile(shape, F32, tag=f"rr_r_{tag}")
    nc.vector.scalar_tensor_tensor(red[:], t[:], -TWO_PI, x_ap, ALU.mult,
                                   ALU.add)
    nc.vector.tensor_scalar(t[:], red[:], PI, None, ALU.is_ge)
    nc.vector.scalar_tensor_tensor(red[:], t[:], -TWO_PI, red[:], ALU.mult,
                                   ALU.add)
    return red


def _rsqrt(nc, small, veps_ap, shape, tag):
    """r = (veps)^-1/2 via bitcast-log2 seed + AF.Exp + 1 Newton step.

    veps_ap must be a positive fp32 AP with contiguous last dim.
    Stays inside the exp table set (no Ln / Rsqrt loads).
    """
    vif = small.tile(shape, F32, tag=f"rs_f_{tag}")
    nc.vector.tensor_copy(vif[:], veps_ap.bitcast(I32))
    nc.vector.tensor_scalar(vif[:], vif[:], RS_SCALE, RS_BIAS, ALU.mult,
                            ALU.add)
    r0 = small.tile(shape, F32, tag=f"rs_r0_{tag}")
    nc.scalar.activation(r0[:], vif[:], AF.Exp, scale=LN2)
    t = small.tile(shape, F32, tag=f"rs_t_{tag}")
    nc.vector.tensor_tensor(t[:], r0[:], r0[:], ALU.mult)
    nc.vector.tensor_tensor(t[:], t[:], veps_ap, ALU.mult)
    nc.vector.tensor_scalar(t[:], t[:], -0.5, 1.5, ALU.mult, ALU.add)
    r = small.tile(shape, F32, tag=f"rs_r_{tag}")
    nc.vector.tensor_tensor(r[:], t[:], r0[:], ALU.mult)
    return r


def _ln_stats(nc, small, src_ap, eps):
    """Free-dim LayerNorm stats of [128, N] fp32 -> (rstd, -mean*rstd)."""
    bnst = small.tile([P, 6], F32, tag="bnst")
    nc.vector.bn_stats(bnst[:], src_ap)
    mv = small.tile([P, 2], F32, tag="mv")
    nc.vector.bn_aggr(mv[:], bnst[:])
    veps = small.tile([P, 1], F32, tag="veps")
    nc.vector.tensor_scalar(veps[:], mv[:, 1:2], 1.0, eps, ALU.mult, ALU.add)
    r = _rsqrt(nc, small, veps[:], [P, 1], "ln")
    mb = small.tile([P, 1], F32, tag="mb")
    nc.vector.scalar_tensor_tensor(mb[:], mv[:, 0:1], -1.0, r[:], ALU.mult,
                                   ALU.mult)
    return r, mb


def _ln_modulate_transpose(nc, small, xn_pool, psum, ident, xc, sc_ap, sh_ap,
                           eps, hhT, tag):
    """LayerNorm(tok-major fp32 xc) -> chan-major, *(1+sc)+sh -> bf16 hhT."""
    xn = xn_pool.tile([P, TC, LAT], BF16, tag="xn_shared")
    for c in range(TC):
        r, mb = _ln_stats(nc, small, xc[:, c, :], eps)
        nc.scalar.activation(xn[:, c, :], xc[:, c, :], AF.Identity,
                             bias=mb[:], scale=r[:])
    for c in range(TC):
        for s in range(LS):
            pst = psum.tile([P, P], BF16, tag="ps")
            nc.tensor.transpose(pst[:], xn[:, c, ts(s, P)], ident[:])
            nc.vector.tensor_scalar(
                hhT[:, s, ts(c, P)], pst[:],
                sc_ap[:, s:s + 1], sh_ap[:, s:s + 1], ALU.mult, ALU.add)


def build_program(depth=DEPTH):
    nc = bacc.Bacc("TRN2", target_bir_lowering=False, debug=False,
                   num_devices=8)

    def din(name, shape, dt):
        return nc.dram_tensor(name, list(shape), dt,
                              kind="ExternalInput").ap()

    x_t = din("x_t", [P, TC, HID], F32)
    coords_t = din("coords_t", [P, TC, 3], F32)
    ident_d = din("ident", [P, P], BF16)
    div_bc_d = din("div_bc", [P, TC, 256], F32)
    cvec_d = din("cvec", [P, 8], F32)
    hsel_d = din("hsel", [2, P], F32)
    projw_d = din("projw", [P, 1, LAT], BF16)
    te1_d = din("te1", [P, 2, LAT], BF16)
    te2_d = din("te2", [P, LS, LAT], BF16)
    adaw_d = din("adaw", [depth, P, LS, 6 * LAT], BF16)
    pew_d = din("pew", [depth, P, LS, 2 * LAT], BF16)     # [p1 | p2]
    qkvw_d = din("qkvw", [depth, P, LS, 3 * LAT], BF16)   # [qk | v]
    pjw_d = din("pjw", [depth, P, LS, LAT], BF16)
    f1w_d = din("f1w", [depth, P, LS * MLP_H], BF16)
    f2w_d = din("f2w", [depth, P, MS * LAT], BF16)
    finada_d = din("finada", [P, LS, 2 * LAT], BF16)
    finw_d = din("finw", [P, LS, OUT_C], BF16)

    out_t = nc.dram_tensor("out_t", [P, TC, OUT_C], F32,
                           kind="ExternalOutput").ap()

    with tile.TileContext(nc) as tc:
        with ExitStack() as ctx:
            _emit(ctx, tc, nc, depth, x_t, coords_t, ident_d,
                  div_bc_d, cvec_d, hsel_d, projw_d, te1_d, te2_d,
                  adaw_d, pew_d, qkvw_d, pjw_d, f1w_d, f2w_d,
                  finada_d, finw_d, out_t)
    nc.compile()
    return nc


def _emit(ctx, tc, nc, depth, x_t, coords_t, ident_d, div_bc_d,
          cvec_d, hsel_d, projw_d, te1_d, te2_d, adaw_d, pew_d, qkvw_d,
          pjw_d, f1w_d, f2w_d, finada_d, finw_d, out_t):
    def pool(name, bufs, space="SBUF"):
        return ctx.enter_context(
            tc.tile_pool(name=name, bufs=bufs, space=space))

    pers = pool("pers", 1)    # persistent state + consts
    once = pool("once", 1)    # prelude / final-layer temporaries
    wA = pool("wA", 1)        # ada weights
    wM = pool("wM", 1)        # pe-mask weights
    wQ = pool("wQ", 1)        # qkv weights
    wP = pool("wP", 1)        # proj weights
    wF1 = pool("wF1", 1)      # fc1 weights
    wF2 = pool("wF2", 1)      # fc2 weights
    blk1 = pool("blk1", 1)    # within-block activations (xc chain)
    blk2 = pool("blk2", 2)    # run-ahead tiles (ada / mask chains)
    small = pool("small", 3)  # tiny stat tiles
    psA = pool("psA", 4, space="PSUM")    # xc-chain matmul banks
    psM = pool("psM", 2, space="PSUM")    # mask-chain banks
    psR = pool("psR", 2, space="PSUM")    # ada row banks
    dram = pool("dram", 3, space="DRAM")

    # ---------------- persistent/consts ----------------
    ident = pers.tile([P, P], BF16, tag="ident")
    nc.sync.dma_start(ident[:], ident_d)
    cvec = pers.tile([P, 8], F32, tag="cvec")
    nc.sync.dma_start(cvec[:], cvec_d)
    hsel = pers.tile([2, P], F32, tag="hsel")
    nc.sync.dma_start(hsel[:], hsel_d)
    sel2 = cvec[:, 5:7]

    xc = pers.tile([P, TC, LAT], F32, tag="xc")
    v_aug = pers.tile([P, TC, HEADS, HD + 1], BF16, tag="v_aug")
    nc.vector.memset(v_aug[:, :, :, HD:HD + 1], 1.0)

    # ---------------- t embedding -> sT = silu(c) chan-major ----------------
    tf = small.tile([P, 1], F32, tag="tf")
    nc.vector.tensor_scalar(tf[:], cvec[:, 3:4], cvec[:, 4:5], None,
                            ALU.mult)
    embT = once.tile([P, 2, 1], BF16, tag="embT")
    for idx, off in ((0, PI / 2.0), (1, 0.0)):  # sub0=cos, sub1=sin
        xsh = small.tile([P, 1], F32, tag="tf_sh")
        nc.vector.tensor_scalar(xsh[:], tf[:], off, None, ALU.add)
        red = _range_reduce(nc, small, xsh[:], [P, 1], "emb")
        nc.scalar.activation(embT[:, idx, :], red[:], AF.Sin)
    te1 = once.tile([P, 2, LAT], BF16, tag="peb")
    nc.sync.dma_start(te1[:], te1_d)
    te2 = once.tile([P, LS, LAT], BF16, tag="pe_slot")
    nc.sync.dma_start(te2[:], te2_d)
    ps_h1 = psR.tile([1, LAT], F32, tag="psa")
    for s in range(2):
        nc.tensor.matmul(ps_h1[:], embT[:, s, :], te1[:, s, :],
                         start=(s == 0), stop=(s == 1))
    h1row = once.tile([1, LAT], BF16, tag="h1row")
    nc.scalar.activation(h1row[:], ps_h1[:], AF.Silu)
    h1_dr = dram.tile([1, LAT], BF16, tag="h1_dr")
    nc.sync.dma_start(h1_dr[:], h1row[:])
    h1T = once.tile([P, LS, 1], BF16, tag="h1T")
    nc.sync.dma_start(h1T[:, :, 0],
                      h1_dr[:].rearrange("o (s p) -> (o p) s", s=LS, p=P))
    ps_c = psR.tile([1, LAT], F32, tag="psa")
    for s in range(LS):
        nc.tensor.matmul(ps_c[:], h1T[:, s, :], te2[:, s, :],
                         start=(s == 0), stop=(s == LS - 1))
    s_row = once.tile([1, LAT], BF16, tag="s_row")
    nc.scalar.activation(s_row[:], ps_c[:], AF.Silu)
    s_dr = dram.tile([1, LAT], BF16, tag="s_dr")
    nc.sync.dma_start(s_dr[:], s_row[:])
    sT = pers.tile([P, LS, 1], BF16, tag="sT")
    nc.sync.dma_start(sT[:, :, 0],
                      s_dr[:].rearrange("o (s p) -> (o p) s", s=LS, p=P))

    # ---------------- positional encoding ----------------
    cds = once.tile([P, TC, 3], F32, tag="cds")
    nc.sync.dma_start(cds[:], coords_t)
    div_bc = once.tile([P, TC, 256], F32, tag="div_bc_slot")
    nc.sync.dma_start(div_bc[:], div_bc_d)
    enc = small.tile([P, TC, 1], F32, tag="enc")
    nc.vector.scalar_tensor_tensor(enc[:, :, 0], cds[:, :, 1], 100.0,
                                   cds[:, :, 2], ALU.mult, ALU.add)
    nc.vector.scalar_tensor_tensor(enc[:, :, 0], cds[:, :, 0], 10000.0,
                                   enc[:, :, 0], ALU.mult, ALU.add)
    ang = once.tile([P, TC, 256], F32, tag="ang")
    nc.vector.tensor_tensor(ang[:], div_bc[:],
                            enc[:].to_broadcast((P, TC, 256)), ALU.mult)
    pe = once.tile([P, TC, LAT], F32, tag="pe_slot")
    pe4 = pe[:].rearrange("p c (j k) -> p c j k", j=256, k=2)
    for k, off in ((0, 0.0), (1, PI / 2.0)):  # even=sin, odd=cos
        xsh = once.tile([P, TC, 256], F32, tag="ang_sh")
        nc.vector.tensor_scalar(xsh[:], ang[:], off, None, ALU.add)
        red = _range_reduce(nc, once, xsh[:], [P, TC, 256], "pe")
        nc.scalar.activation(pe4[:, :, :, k], red[:], AF.Sin)
    peb = once.tile([P, TC, LAT], BF16, tag="peb")
    nc.vector.tensor_copy(peb[:], pe[:])
    peT = pers.tile([P, LS, TOK], BF16, tag="peT")
    for c in range(TC):
        for s in range(LS):
            pst = psA.tile([P, P], BF16, tag="ps")
            nc.tensor.transpose(pst[:], peb[:, c, ts(s, P)], ident[:])
            nc.vector.tensor_copy(peT[:, s, ts(c, P)], pst[:])

    # ---------------- input projection ----------------
    xin = once.tile([P, TC, HID], F32, tag="xin_slot")
    nc.sync.dma_start(xin[:], x_t)
    xn0 = once.tile([P, TC, HID], BF16, tag="xn0")
    for c in range(TC):
        r, mb = _ln_stats(nc, small, xin[:, c, :], 1e-5)
        nc.scalar.activation(xn0[:, c, :], xin[:, c, :], AF.Identity,
                             bias=mb[:], scale=r[:])
    xn0T = once.tile([P, TOK], BF16, tag="xn0T")
    for c in range(TC):
        pst = psA.tile([P, P], BF16, tag="ps")
        nc.tensor.transpose(pst[:], xn0[:, c, :], ident[:])
        nc.vector.tensor_copy(xn0T[:, ts(c, P)], pst[:])
    projw = once.tile([P, 1, LAT], BF16, tag="projw")
    nc.sync.dma_start(projw[:], projw_d)
    for c in range(TC):
        ph = psA.tile([P, LAT], F32, tag="ps")
        nc.tensor.matmul(ph[:], xn0T[:, ts(c, P)], projw[:, 0, :],
                         start=True, stop=True)
        hsb = once.tile([P, LAT], F32, tag="h_sb")
        nc.scalar.activation(hsb[:], ph[:], AF.Copy)
        r, mb = _ln_stats(nc, small, hsb[:], 1e-5)
        hn = once.tile([P, LAT], F32, tag="hn")
        nc.scalar.activation(hn[:], hsb[:], AF.Identity, bias=mb[:],
                             scale=r[:])
        nc.vector.tensor_tensor(xc[:, c, :], hn[:], pe[:, c, :], ALU.add)

    # ---------------- transformer blocks ----------------
    inv_gn = 1.0 / (64.0 * 256.0)

    for d in range(depth):
        # --- weight group DMAs, in use order ---
        adaw = wA.tile([P, LS, 6 * LAT], BF16, tag="adaw")
        nc.sync.dma_start(adaw[:], adaw_d[d])
        pew = wM.tile([P, LS, 2 * LAT], BF16, tag="pew")
        nc.sync.dma_start(pew[:], pew_d[d])
        qkvw = wQ.tile([P, LS, 3 * LAT], BF16, tag="qkvw")
        nc.sync.dma_start(qkvw[:], qkvw_d[d])
        pjw = wP.tile([P, LS, LAT], BF16, tag="pjw")
        nc.sync.dma_start(pjw[:], pjw_d[d])
        f1w = wF1.tile([P, LS * MLP_H], BF16, tag="f1w")
        nc.sync.dma_start(f1w[:], f1w_d[d])
        f2w = wF2.tile([P, MS * LAT], BF16, tag="f2w")
        nc.sync.dma_start(f2w[:], f2w_d[d])
        p1w = pew[:, :, 0:LAT]
        p2w = pew[:, :, LAT:2 * LAT]
        qkw = qkvw[:, :, 0:2 * LAT]
        vw = qkvw[:, :, 2 * LAT:3 * LAT]
        f1v = f1w[:].rearrange("p (s n) -> p s n", s=LS)
        f2v = f2w[:].rearrange("p (s n) -> p s n", s=MS)

        # --- ada = silu(c) @ aw^T : [1, 3072] row, scatter via DRAM ---
        ada_dr = dram.tile([1, 6 * LAT], BF16, tag="ada_dr")
        for n in range(6):
            psa = psR.tile([1, LAT], F32, tag="psa")
            for s in range(LS):
                nc.tensor.matmul(psa[:], sT[:, s, :],
                                 adaw[:, s, ts(n, LAT)],
                                 start=(s == 0), stop=(s == LS - 1))
            arow = blk2.tile([1, LAT], BF16, tag="arow")
            nc.scalar.activation(arow[:], psa[:], AF.Copy)
            nc.sync.dma_start(ada_dr[:, ts(n, LAT)], arow[:])
        adaTb = blk2.tile([P, 6, LS], BF16, tag="adaTb")
        nc.sync.dma_start(
            adaTb[:],
            ada_dr[:].rearrange("o (v s p) -> (o p) v s", v=6, s=LS, p=P))
        adaT = blk2.tile([P, 6, LS], F32, tag="adaT")
        nc.vector.tensor_copy(adaT[:], adaTb[:])
        nc.vector.tensor_scalar(adaT[:, 1, :], adaT[:, 1, :], 1.0, None,
                                ALU.add)
        nc.vector.tensor_scalar(adaT[:, 4, :], adaT[:, 4, :], 1.0, None,
                                ALU.add)
        ga_bc = blk2.tile([P, LAT], BF16, tag="ga_bc")
        nc.sync.dma_start(ga_bc[:],
                          ada_dr[:, ts(2, LAT)].to_broadcast((P, LAT)))
        gm_bc = blk2.tile([P, LAT], BF16, tag="gm_bc")
        nc.sync.dma_start(gm_bc[:],
                          ada_dr[:, ts(5, LAT)].to_broadcast((P, LAT)))

        # --- mask chain: pe1/pe2 matmuls + GN (on-chip reduce) ---
        def pe_branch(w_sb, dst_bf, tagp):
            raw = blk2.tile([P, LS, TOK], BF16, tag=f"pe_raw_{tagp}")
            st1 = small.tile([P, 8], F32, tag=f"gn_st1_{tagp}")
            for m in range(LS):
                psp = psM.tile([P, TOK], F32, tag="psm")
                for s in range(LS):
                    nc.tensor.matmul(psp[:], w_sb[:, s, ts(m, P)],
                                     peT[:, s, :], start=(s == 0),
                                     stop=(s == LS - 1))
                nc.scalar.activation(raw[:, m, :], psp[:], AF.Copy,
                                     accum_out=st1[:, m:m + 1])
                sq = blk1.tile([P, TOK], BF16, tag="gn_sq")
                nc.scalar.activation(sq[:], psp[:], AF.Square,
                                     accum_out=st1[:, 4 + m:5 + m])
            # partition-reduce into [half, (sum4 | sumsq4)]
            gps = psM.tile([2, 8], F32, tag="psm")
            nc.tensor.matmul(gps[:], sel2, st1[:], start=True, stop=True)
            gs2 = small.tile([2, 8], F32, tag=f"gs2_{tagp}")
            nc.vector.tensor_copy(gs2[:], gps[:])
            mu = small.tile([2, 4], F32, tag=f"gn_mu_{tagp}")
            nc.vector.tensor_scalar(mu[:], gs2[:, 0:4], inv_gn, None,
                                    ALU.mult)
            m2e = small.tile([2, 4], F32, tag=f"gn_m2_{tagp}")
            nc.vector.tensor_scalar(m2e[:], gs2[:, 4:8], inv_gn, 1e-5,
                                    ALU.mult, ALU.add)
            var = small.tile([2, 4], F32, tag=f"gn_var_{tagp}")
            nc.vector.tensor_tensor(var[:], mu[:], mu[:], ALU.mult)
            nc.vector.tensor_tensor(var[:], m2e[:], var[:], ALU.subtract)
            r = _rsqrt(nc, small, var[:], [2, 4], f"gn_{tagp}")
            ab = small.tile([2, 4, 2], F32, tag=f"gn_ab_{tagp}")
            nc.vector.tensor_copy(ab[:, :, 0], r[:])
            nc.vector.scalar_tensor_tensor(ab[:, :, 1], mu[:], -1.0, r[:],
                                           ALU.mult, ALU.mult)
            abps = psM.tile([P, 8], F32, tag="psm")
            nc.tensor.matmul(abps[:], hsel[:],
                             ab[:].rearrange("h m j -> h (m j)"),
                             start=True, stop=True)
            abbc = small.tile([P, 8], F32, tag=f"gn_abbc_{tagp}")
            nc.vector.tensor_copy(abbc[:], abps[:])
            for m in range(LS):
                nc.vector.tensor_scalar(
                    dst_bf[:, m, :], raw[:, m, :],
                    abbc[:, 2 * m:2 * m + 1], abbc[:, 2 * m + 1:2 * m + 2],
                    ALU.mult, ALU.add)

        pe1n = blk2.tile([P, LS, TOK], BF16, tag="pe1n")
        pe_branch(p1w, pe1n[:], "p1")
        pe2n = blk2.tile([P, LS, TOK], BF16, tag="pe2n")
        pe_branch(p2w, pe2n[:], "p2")
        # maskT[k, q] = sigmoid(sum_c pe2n[c,k] pe1n[c,q]) via tanh
        maskT = blk2.tile([P, TC, TOK], F32, tag="maskT")
        for mc in range(TC):
            psm = psM.tile([P, TOK], F32, tag="psm")
            for s in range(LS):
                nc.tensor.matmul(psm[:], pe2n[:, s, ts(mc, P)],
                                 pe1n[:, s, :], start=(s == 0),
                                 stop=(s == LS - 1))
            nc.scalar.activation(maskT[:, mc, :], psm[:], AF.Tanh,
                                 scale=0.5)
        nc.vector.tensor_scalar(maskT[:], maskT[:], 0.5, 0.5, ALU.mult,
                                ALU.add)

        # --- LN1 + modulate + transpose ---
        hh1T = blk1.tile([P, LS, TOK], BF16, tag="hh1T")
        _ln_modulate_transpose(nc, small, blk1, psA, ident, xc[:],
                               adaT[:, 1, :], adaT[:, 0, :], 1e-6, hh1T[:],
                               "l1")

        # --- qk^T (chan-major, pair-packed psum) ---
        qkT = blk1.tile([P, 2 * LS, TOK], BF16, tag="qkT")
        for mp in range(LS):
            psq = psA.tile([P, 2, TOK], F32, tag="ps")
            for half in range(2):
                m = 2 * mp + half
                for s in range(LS):
                    nc.tensor.matmul(psq[:, half, :], qkw[:, s, ts(m, P)],
                                     hh1T[:, s, :], start=(s == 0),
                                     stop=(s == LS - 1))
            nc.vector.tensor_copy(qkT[:, 2 * mp:2 * mp + 2, :], psq[:])
        # --- v (tok-major, ones col persists in v_aug) ---
        for c in range(TC):
            psv = psA.tile([P, LAT], F32, tag="ps")
            for s in range(LS):
                nc.tensor.matmul(psv[:], hh1T[:, s, ts(c, P)], vw[:, s, :],
                                 start=(s == 0), stop=(s == LS - 1))
            nc.scalar.activation(
                v_aug[:, c, :, 0:HD],
                psv[:].rearrange("p (h d) -> p h d", h=HEADS), AF.Copy)

        # --- attention, key-major scores ---
        stf = blk1.tile([P, HEADS, TC, TOK], BF16, tag="stf")
        for h in range(HEADS):
            pbase = (h % 2) * HD
            qs = qkT[pbase:pbase + HD, h // 2, :]
            ks = qkT[pbase:pbase + HD, 4 + h // 2, :]
            pss = psA.tile([P, TC, TOK], F32, tag="ps")
            for kc in range(TC):
                nc.tensor.matmul(pss[:, kc, :], ks[:, ts(kc, P)], qs,
                                 start=True, stop=True)
            nc.vector.tensor_tensor(stf[:, h], pss[:], maskT[:], ALU.mult)
        ptil = blk1.tile([P, HEADS, TC, TOK], BF16, tag="ptil")
        for hq in range(2):
            nc.scalar.activation(ptil[:, ts(hq, 4)], stf[:, ts(hq, 4)],
                                 AF.Exp)
        attn = blk1.tile([P, TC, LAT], BF16, tag="attn")
        for qc in range(TC):
            for g in range(2):
                po = psA.tile([P, 4, HD + 1], F32, tag="ps")
                for j in range(4):
                    h = 4 * g + j
                    for kc in range(TC):
                        nc.tensor.matmul(po[:, j, :],
                                         ptil[:, h, kc, ts(qc, P)],
                                         v_aug[:, kc, h, :],
                                         start=(kc == 0),
                                         stop=(kc == TC - 1))
                rz = small.tile([P, 4, 1], F32, tag="rz")
                nc.vector.reciprocal_approx_fast(
                    out=rz[:], in_=po[:, :, HD:HD + 1])
                nc.vector.tensor_tensor(
                    attn[:, qc, ts(g, 256)].rearrange(
                        "p (j d) -> p j d", d=HD),
                    po[:, :, 0:HD], rz[:].to_broadcast((P, 4, HD)),
                    ALU.mult)
        attnT = blk1.tile([P, LS, TOK], BF16, tag="attnT")
        for c in range(TC):
            for s in range(LS):
                pst = psA.tile([P, P], BF16, tag="ps")
                nc.tensor.transpose(pst[:], attn[:, c, ts(s, P)], ident[:])
                nc.vector.tensor_copy(attnT[:, s, ts(c, P)], pst[:])

        # --- attn proj + gated residual ---
        for c in range(TC):
            psp = psA.tile([P, LAT], F32, tag="ps")
            for s in range(LS):
                nc.tensor.matmul(psp[:], attnT[:, s, ts(c, P)], pjw[:, s, :],
                                 start=(s == 0), stop=(s == LS - 1))
            gated = blk1.tile([P, LAT], F32, tag="gated")
            nc.vector.tensor_tensor(gated[:], psp[:], ga_bc[:], ALU.mult)
            nc.vector.tensor_tensor(xc[:, c, :], xc[:, c, :], gated[:],
                                    ALU.add)

        # --- LN2 + modulate + transpose ---
        hh2T = blk1.tile([P, LS, TOK], BF16, tag="hh2T")
        _ln_modulate_transpose(nc, small, blk1, psA, ident, xc[:],
                               adaT[:, 4, :], adaT[:, 3, :], 1e-6, hh2T[:],
                               "l2")

        # --- MLP ---
        mlpT = blk1.tile([P, MS, TOK], BF16, tag="mlpT")
        for mp in range(MS // 2):
            psf = psA.tile([P, 2, TOK], F32, tag="ps")
            for half in range(2):
                m = 2 * mp + half
                for s in range(LS):
                    nc.tensor.matmul(psf[:, half, :], f1v[:, s, ts(m, P)],
                                     hh2T[:, s, :], start=(s == 0),
                                     stop=(s == LS - 1))
            nc.scalar.activation(mlpT[:, 2 * mp:2 * mp + 2, :], psf[:],
                                 AF.Gelu)
        for c in range(TC):
            psm2 = psA.tile([P, LAT], F32, tag="ps")
            for s in range(MS):
                nc.tensor.matmul(psm2[:], mlpT[:, s, ts(c, P)], f2v[:, s, :],
                                 start=(s == 0), stop=(s == MS - 1))
            gated = blk1.tile([P, LAT], F32, tag="gated")
            nc.vector.tensor_tensor(gated[:], psm2[:], gm_bc[:], ALU.mult)
            nc.vector.tensor_tensor(xc[:, c, :], xc[:, c, :], gated[:],
                                    ALU.add)

    # ---------------- final layer ----------------
    finada = wA.tile([P, LS, 2 * LAT], BF16, tag="adaw")
    nc.sync.dma_start(finada[:], finada_d)
    finw = wQ.tile([P, LS, OUT_C], BF16, tag="qkvw")
    nc.sync.dma_start(finw[:], finw_d)
    adaf_dr = dram.tile([1, 2 * LAT], F32, tag="adaf_dr")
    afrow = once.tile([1, 2 * LAT], F32, tag="xin_slot")
    for n in range(2):
        psa = psR.tile([1, LAT], F32, tag="psa")
        for s in range(LS):
            nc.tensor.matmul(psa[:], sT[:, s, :], finada[:, s, ts(n, LAT)],
                             start=(s == 0), stop=(s == LS - 1))
        nc.scalar.activation(afrow[:, ts(n, LAT)], psa[:], AF.Copy)
    nc.sync.dma_start(adaf_dr[:], afrow[:])
    adaTf = once.tile([P, 2, LS], F32, tag="adaTf")
    nc.sync.dma_start(
        adaTf[:],
        adaf_dr[:].rearrange("o (v s p) -> (o p) v s", v=2, s=LS, p=P))
    nc.vector.tensor_scalar(adaTf[:, 1, :], adaTf[:, 1, :], 1.0, None,
                            ALU.add)
    hhfT = once.tile([P, LS, TOK], BF16, tag="ang")
    _ln_modulate_transpose(nc, small, once, psA, ident, xc[:],
                           adaTf[:, 1, :], adaTf[:, 0, :], 1e-6, hhfT[:],
                           "lf")
    outsb = once.tile([P, TC, OUT_C], F32, tag="div_bc_slot")
    for mc in range(TC):
        pso = psA.tile([P, OUT_C], F32, tag="ps")
        for s in range(LS):
            nc.tensor.matmul(pso[:], finw[:, s, ts(mc, P)], hhfT[:, s, :],
                             start=(s == 0), stop=(s == LS - 1))
        nc.scalar.activation(outsb[:, mc, :], pso[:], AF.Copy)
    nc.sync.dma_start(out_t, outsb[:])


# ---------------------------------------------------------------------------
# host side
# ---------------------------------------------------------------------------

def _to_bf16(a):
    return np.asarray(a, dtype=np.float32).astype(ml_dtypes.bfloat16)


def _stage_wT(w, S, N):
    """w: [..., N_out, K] -> W^T staged [..., 128, S, N_out] bf16."""
    wt = np.ascontiguousarray(np.swapaxes(np.asarray(w, np.float32), -1, -2))
    shp = wt.shape
    K, M = shp[-2], shp[-1]
    assert K == S * P and M == N, (shp, S, N)
    wt = wt.reshape(shp[:-2] + (S, P, M))
    wt = np.swapaxes(wt, -3, -2)  # [..., P, S, M]
    return _to_bf16(np.ascontiguousarray(wt))


_CACHE = {}


def _get_program():
    if DEPTH not in _CACHE:
        _CACHE[DEPTH] = build_program(DEPTH)
    return _CACHE[DEPTH]


def prepare_in_maps(inputs, depth=DEPTH):
    f32 = np.float32
    x = np.asarray(inputs["x"], f32)
    t = np.asarray(inputs["t"], f32)
    coords = np.asarray(inputs["coords"], f32)

    qkv = np.asarray(inputs["blk_qkv_w"], f32)[:depth]
    qk = qkv[:, :2 * LAT, :].copy()
    qk[:, :LAT, :] *= HD ** -0.5
    vw = qkv[:, 2 * LAT:, :]

    hsel = np.zeros((2, P), f32)
    hsel[0, 0:64] = 1.0
    hsel[1, 64:128] = 1.0

    common = {
        "ident": _to_bf16(np.eye(P)),
        "div_bc": np.ascontiguousarray(np.broadcast_to(
            np.exp(np.arange(0, LAT, 2, dtype=f32)
                   * (-math.log(10000.0) / LAT)),
            (P, TC, 256)).astype(f32)),
        "hsel": hsel,
        "projw": _stage_wT(inputs["proj_w"], 1, LAT),
        "te1": _stage_wT(inputs["te_w1"], 2, LAT),
        "te2": _stage_wT(inputs["te_w2"], LS, LAT),
        "adaw": _stage_wT(np.asarray(inputs["blk_ada_w"], f32)[:depth],
                          LS, 6 * LAT),
        "pew": np.concatenate([
            _stage_wT(np.asarray(inputs["blk_pe1_w"], f32)[:depth], LS, LAT),
            _stage_wT(np.asarray(inputs["blk_pe2_w"], f32)[:depth], LS, LAT),
        ], axis=3),
        "qkvw": np.concatenate([
            _stage_wT(qk, LS, 2 * LAT),
            _stage_wT(vw, LS, LAT),
        ], axis=3),
        "pjw": _stage_wT(np.asarray(inputs["blk_proj_w"], f32)[:depth],
                         LS, LAT),
        "f1w": _stage_wT(np.asarray(inputs["blk_fc1_w"], f32)[:depth],
                         LS, MLP_H).reshape(depth, P, LS * MLP_H),
        "f2w": _stage_wT(np.asarray(inputs["blk_fc2_w"], f32)[:depth],
                         MS, LAT).reshape(depth, P, MS * LAT),
        "finada": _stage_wT(inputs["fin_ada_w"], LS, 2 * LAT),
        "finw": _stage_wT(inputs["fin_w"], LS, OUT_C),
    }
    in_maps = []
    for b in range(B):
        m = dict(common)
        m["x_t"] = np.ascontiguousarray(
            x[b].T.reshape(TC, P, HID).swapaxes(0, 1))
        m["coords_t"] = np.ascontiguousarray(
            coords[b].T.reshape(TC, P, 3).swapaxes(0, 1))
        cv = np.zeros((P, 8), f32)
        cv[:, 0] = -math.pi
        cv[:, 1] = 1e-5
        cv[:, 2] = 1e-6
        cv[:, 3] = np.exp(-math.log(10000.0)
                          * np.arange(FREQ // 2, dtype=f32) / (FREQ // 2))
        cv[:, 4] = t[b]
        cv[:, 5] = (np.arange(P) // 64 == 0)
        cv[:, 6] = (np.arange(P) // 64 == 1)
        m["cvec"] = cv
        in_maps.append(m)
    return in_maps


def run_spmd(inputs, **kw):
    nc = _get_program()
    in_maps = prepare_in_maps(inputs, DEPTH)
    res = run_bass_kernel_spmd(nc, in_maps, core_ids=list(range(B)), **kw)
    outs = []
    for b in range(B):
        o = np.asarray(res.results[b]["out_t"], np.float32)  # [P, TC, OUT]
        outs.append(o.transpose(1, 0, 2).reshape(OUT_C, TOK))
    return np.stack(outs, axis=0), res


def kernel(**inputs):
    out, _ = run_spmd(inputs)
    return out
